# revision 45
# baseline (speedup 1.0000x reference)
"""Trainium2 Bass kernel: tridiagonal solve A(alpha) x = f, N = 4M, f32.

v3: interleaved dual-stream scans at 1 elem/cycle + fused pivot/reciprocal
+ per-chunk padded layouts (no cross-chunk WAR serialization) + per-chunk
fwd->bwd pipelining (output DMA starts ~1/4 into the compute).

Core trick (v2): the stock DVE tensor_tensor_scan costs 2 cycles/element
(bubble covering the stage-(d+1) -> stage-d feedback).  A hand-written uOp
program (SCAN2A/SCAN2S) with NO bubble computes the stride-2 recurrence
    out[c] = in0[c] * out[c-2] (+/-) in1[c]     (state seeds at 0)
at 1 element/cycle; each lane's 4096 rows are split into two independent
2048-row halves (diagonal dominance: fwd gain <=0.097/row, bwd <=0.74/row
=> halo warmup decouples them) interleaved column-wise on the host.

v3 additions:
- M0RN custom op: rn = -1/m0 via 2-term Neumann  rn ~= (m0-1) - 1, fused
  into the pivot polynomial: rn = a*(a^2-(a+2)*b^2) - 1.  Error e^2 <=
  7.4e-4 relative -- removes the ACT Reciprocal pass entirely.
- every intermediate lives in per-chunk halo-extended blocks, written
  exactly once (no overlapping writes -> no cross-engine WAR stalls).
- fwd chunk scans extend +HBW cols so bwd of the SAME chunk has its
  warmup spill locally: per-chunk pipeline m0rn->A,Bn->fwd->ncp->bwd->DMA.
- output x in fp16 (halves the out DMA); all DMA on the sync+scalar HW
  DGE queues (gpsimd SWDGE drains cost ~4us at the tail -- avoided).
"""

import contextlib

import numpy as np

import concourse.bacc as bacc
import concourse.bass as bass
import concourse.mybir as mybir
import concourse.tile as tile

import numpy as _np
from concourse import dve_ops as _dvo
from concourse.dve_spec import Spec as _Spec, Src0 as _S0, Src1 as _S1, One as _One
from concourse.dve_spec import lower as _dve_lower, _has_src1
from concourse.dve_uop import (
    DveOpSpec as _DveOpSpec,
    UopConfig as _UopConfig,
    AluOp as _UAluOp,
    AluInp as _AluInp,
    InpSel as _InpSel,
    OutSel as _OutSel,
    OutPath as _OutPath,
    Trigger as _Trigger,
    DelayInp as _DelayInp,
    ENABLE as _ENABLE,
)

N = 4_194_304
NCORES = 8
P = 128
D = N // (NCORES * P)   # 4096 rows per lane
SCAN_2X = True          # 2-elem/cycle scans: 4 interleaved streams + 2x uOps
S = 4 if SCAN_2X else 2  # interleaved streams per lane
DH = D // S             # rows per stream
HF = 8                  # forward warmup halo (rows)
HB = 16                 # backward warmup halo (rows)
W = HF + DH + HB        # f-window rows per stream
WA = W + 2              # alpha window rows per stream (+-1 shifts)
T2 = S * W              # f/compute tile cols (interleaved)
TA2 = S * WA            # alpha tile cols
HFW = S * HF            # fwd warmup cols
HBW = S * HB            # bwd warmup cols
F32 = mybir.dt.float32
F16 = mybir.dt.float16
ALU = mybir.AluOpType
ACTF = mybir.ActivationFunctionType

# ---- tunables -------------------------------------------------------------
# chunk size fractions of T2 (must sum to 1); graduated so chunk arrival
# (input-DMA-paced) tracks the compute ramp
FRACS = [0.10, 0.16, 0.22, 0.26, 0.26]
NH = len(FRACS)
POOL_PRODUCTS = False  # measured: Pool tt costs +10us wall (contention + Q7 speed)


def _register_dve_op(name, spec, subdim=False, uops=None, uops_2x=None):
    existing = {op.name: op for op in _dvo.OPS}
    if name in existing:
        return existing[name]
    row = max(_dvo._SUB_OPCODE_FOR_NAME.values()) + 1
    assert row < 0x20
    shas = {}
    for ver in ("v3", "v4"):
        compiled = _DveOpSpec(
            name=name,
            opcode=row,
            uops=uops if uops is not None else _dve_lower(spec, ver=ver),
            uops_2x=uops_2x,
            perf_max=1 if uops_2x is not None else 0,
            rd1_en=_has_src1(spec),
        )
        shas[ver] = compiled.sha(ver)
        _dvo._COMPILE_CACHE[(name, ver)] = compiled
    op = _dvo.DveOp(name, spec, subdim=subdim, uops_sha=shas)
    _dvo.OPS.append(op)
    _dvo._SUB_OPCODE_FOR_NAME[name] = row
    _dvo.CUSTOM_DVE_SPECS[name] = spec
    return op


def _ref_m0rn(in0, in1, c0, c1, c2):
    a = in0.astype(_np.float32)
    b = in1.astype(_np.float32)
    return (a * (a * a - (a + 2.0) * (b * b)) - 1.0).astype(_np.float32)


# rn = -1/m0 ~= (m0 - 2) = e - 1 with e = m0-1 = a*(a^2 - (a+2)*b^2);
# a = alpha[g], b = alpha[g-1].  |e| <= 0.027 so the 2-term Neumann error
# e^2/(1+e) <= 7.4e-4 relative.
OP_M0RN = _register_dve_op(
    "TRIDIAG_M0RN",
    _Spec(
        body=_S0 * ((_S0 * _S0) - ((_S0 + _One) + _One) * (_S1 * _S1)) - _One,
        reference=_ref_m0rn,
    ),
)


def _scan2_uops(subtract):
    """Stride-2 affine scan at 1 elem/cycle: out[c] = in0[c]*out[c-2] +/- in1[c].

    blk0: prod = A (delay ch0) * NEXT_ALU_OUT_A (blk1's a-flop = state of
    this column's parity stream, written 2 pipeline slots earlier).  blk1:
    state' = prod op in1 (delay ch1), latched into blk1's a-flop.  The seed
    uOp issues 2 non-consuming elements writing a-flop := 0, so both
    streams start at 0 and element 0/1 never read stale flop contents."""
    seed = _UopConfig()
    seed.enable_input(_InpSel.SRC_0, 1)
    seed.enable_input(_InpSel.SRC_1, 2)
    seed.enable_input(_InpSel.ZERO, 3)
    bs = seed.datapath_config
    bs[0].enable_alu(_UAluOp.BYPASS, _AluInp.PREV_DELAY_2)
    bs[0].pass_through_delay(0, 1, 2)
    bs[1].enable_alu(_UAluOp.BYPASS, _AluInp.PREV_DELAY_2)
    bs[1].alu_out_a_enable = _ENABLE
    bs[1].pass_through_delay(0, 1, 2)
    for k in range(2, 8):
        bs[k].pass_through_alu()
        bs[k].pass_through_delay(0, 1, 2)
    seed.repeat_count = 2
    seed.trigger = (_Trigger.COUNT, _Trigger.NONE, _Trigger.NONE)
    seed.next_uop = (1, 0, 0)

    st = _UopConfig()
    st.enable_input(_InpSel.SRC_0, 1)
    st.enable_input(_InpSel.SRC_1, 2)
    st.enable_input(_InpSel.ZERO, 3)
    bb = st.datapath_config
    bb[0].enable_alu(_UAluOp.MULTIPLY, _AluInp.PREV_DELAY_0, _AluInp.NEXT_ALU_OUT_A)
    bb[0].pass_through_delay(0, 1, 2)
    bb[1].enable_alu(
        _UAluOp.SUBTRACT if subtract else _UAluOp.ADD,
        _AluInp.PREV_ALU_OUT,
        _AluInp.PREV_DELAY_1,
    )
    bb[1].alu_out_a_enable = _ENABLE
    bb[1].pass_through_delay(0, 1, 2)
    for k in range(2, 8):
        bb[k].pass_through_alu()
        bb[k].pass_through_delay(0, 1, 2)
    st.require_inp0 = _ENABLE
    st.require_inp1 = _ENABLE
    st.enable_output(_OutSel.ALU_OUT, _OutPath.WR0_LO)
    st.trigger = (_Trigger.SRC_TENSOR_DONE, _Trigger.NONE, _Trigger.NONE)
    st.next_uop = (0, 0, 0)
    return [seed, st]


def _ref_scan2(subtract):
    def ref(in0, in1, c0, c1, c2):
        A = _np.asarray(in0, dtype=_np.float32)
        B = _np.asarray(in1, dtype=_np.float32)
        out = _np.empty(B.shape, dtype=_np.float32)
        L = B.shape[-1]
        st = [_np.zeros(B.shape[0], dtype=_np.float32) for _ in range(S)]
        sgn = -1.0 if subtract else 1.0
        for c in range(L):
            p = c % S
            st[p] = A[:, c] * st[p] + sgn * B[:, c]
            out[:, c] = st[p]
        return out

    return ref


def _scan2x_uops(subtract):
    """2x (pair-per-cycle) stride-4 affine scan.  Element pairs (lo, hi)
    carry 4 interleaved streams: lo streams alternate between blk1's
    a-flop (even pairs, steadyA) and b-flop (odd pairs, steadyB); hi
    streams likewise at blk4.  x_lo rides delay chain 5 to WR0_LO; x_hi
    exits via the blk5-7 bypass chain to WR0_HI.

    Returns (uops_regular, uops_2x), both 3 states (table-gen requires
    equal state counts).  The regular program is the stride-2 variant
    padded with a bubble state -- it would be semantically wrong for the
    4-way layout, but the APs always qualify for 2x (fp16, stride +-1,
    4B-aligned); a silent fallback would show up as a large rel_err."""
    op1 = _UAluOp.SUBTRACT if subtract else _UAluOp.ADD

    def wire(u):
        u.enable_input(_InpSel.SRC_0, 1)     # ch0 = A_lo
        u.enable_input(_InpSel.SRC_1, 2)     # ch1 = B_lo
        u.enable_input(_InpSel.SRC_0_HI, 3)  # ch2 = A_hi
        u.enable_input(_InpSel.SRC_1_HI, 4)  # ch3 = B_hi
        u.enable_input(_InpSel.ZERO, 5)      # ch4 = 0
        return u

    seed = wire(_UopConfig())
    bs = seed.datapath_config
    bs[0].pass_through_alu()
    bs[0].pass_through_delay(0, 1, 2, 3, 4)
    bs[1].enable_alu(_UAluOp.BYPASS, _AluInp.PREV_DELAY_4)
    bs[1].alu_out_a_enable = _ENABLE
    bs[1].alu_out_b_enable = _ENABLE
    bs[1].pass_through_delay(0, 1, 2, 3, 4)
    bs[2].pass_through_alu()
    bs[2].pass_through_delay(2, 3, 4)
    bs[3].pass_through_alu()
    bs[3].pass_through_delay(2, 3, 4)
    bs[4].enable_alu(_UAluOp.BYPASS, _AluInp.PREV_DELAY_4)
    bs[4].alu_out_a_enable = _ENABLE
    bs[4].alu_out_b_enable = _ENABLE
    for k in range(5, 8):
        bs[k].pass_through_alu()
    seed.repeat_count = 2
    seed.trigger = (_Trigger.COUNT, _Trigger.NONE, _Trigger.NONE)
    seed.next_uop = (1, 0, 0)

    def steady(phase):
        st = wire(_UopConfig())
        rd = _AluInp.NEXT_ALU_OUT_A if phase == 0 else _AluInp.NEXT_ALU_OUT_B
        bb = st.datapath_config
        bb[0].enable_alu(_UAluOp.MULTIPLY, _AluInp.PREV_DELAY_0, rd)
        bb[0].pass_through_delay(0, 1, 2, 3)
        bb[1].enable_alu(op1, _AluInp.PREV_ALU_OUT, _AluInp.PREV_DELAY_1)
        if phase == 0:
            bb[1].alu_out_a_enable = _ENABLE
        else:
            bb[1].alu_out_b_enable = _ENABLE
        bb[1].pass_through_delay(2, 3)
        bb[2].pass_through_alu()
        bb[2].pass_through_delay(2, 3)
        bb[2].enable_delay_from_src(_DelayInp.PREV_ALU_OUT, 5)  # ch5 = x_lo
        bb[3].enable_alu(_UAluOp.MULTIPLY, _AluInp.PREV_DELAY_2, rd)
        bb[3].pass_through_delay(3, 5)
        bb[4].enable_alu(op1, _AluInp.PREV_ALU_OUT, _AluInp.PREV_DELAY_3)
        if phase == 0:
            bb[4].alu_out_a_enable = _ENABLE
        else:
            bb[4].alu_out_b_enable = _ENABLE
        bb[4].pass_through_delay(5)
        for k in range(5, 8):
            bb[k].pass_through_alu()
            bb[k].pass_through_delay(5)
        st.require_inp0 = _ENABLE
        st.require_inp1 = _ENABLE
        st.enable_output(_OutSel.DELAY_5, _OutPath.WR0_LO)
        st.enable_output(_OutSel.ALU_OUT, _OutPath.WR0_HI)
        st.repeat_count = 1
        st.trigger = (_Trigger.SRC_TENSOR_DONE, _Trigger.COUNT, _Trigger.NONE)
        st.next_uop = (0, 2 if phase == 0 else 1, 0)
        return st

    uops_2x = [seed, steady(0), steady(1)]

    # regular fallback: stride-2 program padded to 3 states with a bubble
    s2 = _scan2_uops(subtract)
    bubble = _UopConfig()
    bubble.repeat_count = 1
    bubble.trigger = (_Trigger.COUNT, _Trigger.NONE, _Trigger.NONE)
    bubble.next_uop = (2, 0, 0)
    reg = [s2[0], bubble, s2[1]]
    return reg, uops_2x


if SCAN_2X:
    _ra, _r2a = _scan2x_uops(False)
    _rs, _r2s = _scan2x_uops(True)
    OP_SCANA = _register_dve_op(
        "TRIDIAG_SCAN4A",
        _Spec(body=_S0 * _S1, reference=_ref_scan2(False)),
        uops=_ra,
        uops_2x=_r2a,
    )
    OP_SCANS = _register_dve_op(
        "TRIDIAG_SCAN4S",
        _Spec(body=_S0 - _S1, reference=_ref_scan2(True)),
        uops=_rs,
        uops_2x=_r2s,
    )
else:
    OP_SCANA = _register_dve_op(
        "TRIDIAG_SCAN2A",
        _Spec(body=_S0 * _S1, reference=_ref_scan2(False)),
        uops=_scan2_uops(False),
    )
    OP_SCANS = _register_dve_op(
        "TRIDIAG_SCAN2S",
        _Spec(body=_S0 - _S1, reference=_ref_scan2(True)),
        uops=_scan2_uops(True),
    )


_AI = _AluInp


def _asq_uops():
    """A = alpha^2 * rn, 2x pair-per-cycle.  Channels: 0=a_lo 1=rn_lo
    2=a_hi 3=rn_hi 5=park(A_lo)."""
    def base(hi):
        u = _UopConfig()
        u.enable_input(_InpSel.SRC_0, 1)
        u.enable_input(_InpSel.SRC_1, 2)
        if hi:
            u.enable_input(_InpSel.SRC_0_HI, 3)
            u.enable_input(_InpSel.SRC_1_HI, 4)
        u.require_inp0 = _ENABLE
        u.require_inp1 = _ENABLE
        u.trigger = (_Trigger.SRC_TENSOR_DONE, _Trigger.NONE, _Trigger.NONE)
        u.next_uop = (0, 0, 0)
        return u

    reg = base(False)
    rb = reg.datapath_config
    rb[0].enable_alu(_UAluOp.MULTIPLY, _AI.PREV_DELAY_0, _AI.PREV_DELAY_0)
    rb[0].pass_through_delay(1)
    rb[1].enable_alu(_UAluOp.MULTIPLY, _AI.PREV_ALU_OUT, _AI.PREV_DELAY_1)
    for k in range(2, 8):
        rb[k].pass_through_alu()
    reg.enable_output(_OutSel.ALU_OUT, _OutPath.WR0_LO)

    x2 = base(True)
    xb = x2.datapath_config
    xb[0].enable_alu(_UAluOp.MULTIPLY, _AI.PREV_DELAY_0, _AI.PREV_DELAY_0)
    xb[0].pass_through_delay(1, 2, 3)
    xb[1].enable_alu(_UAluOp.MULTIPLY, _AI.PREV_ALU_OUT, _AI.PREV_DELAY_1)
    xb[1].pass_through_delay(2, 3)
    xb[2].pass_through_alu()
    xb[2].pass_through_delay(2, 3)
    xb[2].enable_delay_from_src(_DelayInp.PREV_ALU_OUT, 5)  # park A_lo
    xb[3].enable_alu(_UAluOp.MULTIPLY, _AI.PREV_DELAY_2, _AI.PREV_DELAY_2)
    xb[3].pass_through_delay(3, 5)
    xb[4].enable_alu(_UAluOp.MULTIPLY, _AI.PREV_ALU_OUT, _AI.PREV_DELAY_3)
    xb[4].pass_through_delay(5)
    for k in range(5, 8):
        xb[k].pass_through_alu()
        xb[k].pass_through_delay(5)
    x2.enable_output(_OutSel.DELAY_5, _OutPath.WR0_LO)
    x2.enable_output(_OutSel.ALU_OUT, _OutPath.WR0_HI)
    return [reg], [x2]


def _ncpf_uops():
    """ncp = ((alpha+2)*alpha)*rn, 2x.  Channels: 0=a_lo 1=rn_lo 2=a_hi
    3=rn_hi 4=CONST_0(2.0) 5=park(ncp_lo)."""
    def base(hi):
        u = _UopConfig()
        u.enable_input(_InpSel.SRC_0, 1)
        u.enable_input(_InpSel.SRC_1, 2)
        if hi:
            u.enable_input(_InpSel.SRC_0_HI, 3)
            u.enable_input(_InpSel.SRC_1_HI, 4)
        u.enable_input(_InpSel.CONST_0, 5)
        u.require_inp0 = _ENABLE
        u.require_inp1 = _ENABLE
        u.trigger = (_Trigger.SRC_TENSOR_DONE, _Trigger.NONE, _Trigger.NONE)
        u.next_uop = (0, 0, 0)
        return u

    reg = base(False)
    rb = reg.datapath_config
    rb[0].enable_alu(_UAluOp.ADD, _AI.PREV_DELAY_0, _AI.PREV_DELAY_4)
    rb[0].pass_through_delay(0, 1)
    rb[1].enable_alu(_UAluOp.MULTIPLY, _AI.PREV_ALU_OUT, _AI.PREV_DELAY_0)
    rb[1].pass_through_delay(1)
    rb[2].enable_alu(_UAluOp.MULTIPLY, _AI.PREV_ALU_OUT, _AI.PREV_DELAY_1)
    for k in range(3, 8):
        rb[k].pass_through_alu()
    reg.enable_output(_OutSel.ALU_OUT, _OutPath.WR0_LO)

    x2 = base(True)
    xb = x2.datapath_config
    xb[0].enable_alu(_UAluOp.ADD, _AI.PREV_DELAY_0, _AI.PREV_DELAY_4)
    xb[0].pass_through_delay(0, 1, 2, 3, 4)
    xb[1].enable_alu(_UAluOp.MULTIPLY, _AI.PREV_ALU_OUT, _AI.PREV_DELAY_0)
    xb[1].pass_through_delay(1, 2, 3, 4)
    xb[2].enable_alu(_UAluOp.MULTIPLY, _AI.PREV_ALU_OUT, _AI.PREV_DELAY_1)
    xb[2].pass_through_delay(2, 3, 4)
    # blk3: hi stage 1 + park ncp_lo (blk2's out) on ch5
    xb[3].enable_alu(_UAluOp.ADD, _AI.PREV_DELAY_2, _AI.PREV_DELAY_4)
    xb[3].pass_through_delay(2, 3)
    xb[3].enable_delay_from_src(_DelayInp.PREV_ALU_OUT, 5)
    xb[4].enable_alu(_UAluOp.MULTIPLY, _AI.PREV_ALU_OUT, _AI.PREV_DELAY_2)
    xb[4].pass_through_delay(3, 5)
    xb[5].enable_alu(_UAluOp.MULTIPLY, _AI.PREV_ALU_OUT, _AI.PREV_DELAY_3)
    xb[5].pass_through_delay(5)
    for k in range(6, 8):
        xb[k].pass_through_alu()
        xb[k].pass_through_delay(5)
    x2.enable_output(_OutSel.DELAY_5, _OutPath.WR0_LO)
    x2.enable_output(_OutSel.ALU_OUT, _OutPath.WR0_HI)
    return [reg], [x2]


def _qsu_uops():
    """q = (alpha_g + 2) * alpha_{g-1}^2, 2x.  Channels: 0=a_lo 1=b_lo
    2=a_hi 3=b_hi 4=CONST_0(2.0) 5=park."""
    def base(hi):
        u = _UopConfig()
        u.enable_input(_InpSel.SRC_0, 1)
        u.enable_input(_InpSel.SRC_1, 2)
        if hi:
            u.enable_input(_InpSel.SRC_0_HI, 3)
            u.enable_input(_InpSel.SRC_1_HI, 4)
        u.enable_input(_InpSel.CONST_0, 5)
        u.require_inp0 = _ENABLE
        u.require_inp1 = _ENABLE
        u.trigger = (_Trigger.SRC_TENSOR_DONE, _Trigger.NONE, _Trigger.NONE)
        u.next_uop = (0, 0, 0)
        return u

    reg = base(False)
    rb = reg.datapath_config
    rb[0].enable_alu(_UAluOp.ADD, _AI.PREV_DELAY_0, _AI.PREV_DELAY_4)
    rb[0].pass_through_delay(1)
    rb[1].enable_alu(_UAluOp.MULTIPLY, _AI.PREV_ALU_OUT, _AI.PREV_DELAY_1)
    rb[1].pass_through_delay(1)
    rb[2].enable_alu(_UAluOp.MULTIPLY, _AI.PREV_ALU_OUT, _AI.PREV_DELAY_1)
    for k in range(3, 8):
        rb[k].pass_through_alu()
    reg.enable_output(_OutSel.ALU_OUT, _OutPath.WR0_LO)

    x2 = base(True)
    xb = x2.datapath_config
    xb[0].enable_alu(_UAluOp.ADD, _AI.PREV_DELAY_0, _AI.PREV_DELAY_4)
    xb[0].pass_through_delay(1, 2, 3, 4)
    xb[1].enable_alu(_UAluOp.MULTIPLY, _AI.PREV_ALU_OUT, _AI.PREV_DELAY_1)
    xb[1].pass_through_delay(1, 2, 3, 4)
    xb[2].enable_alu(_UAluOp.MULTIPLY, _AI.PREV_ALU_OUT, _AI.PREV_DELAY_1)
    xb[2].pass_through_delay(2, 3, 4)
    xb[3].enable_alu(_UAluOp.ADD, _AI.PREV_DELAY_2, _AI.PREV_DELAY_4)
    xb[3].pass_through_delay(3)
    xb[3].enable_delay_from_src(_DelayInp.PREV_ALU_OUT, 5)
    xb[4].enable_alu(_UAluOp.MULTIPLY, _AI.PREV_ALU_OUT, _AI.PREV_DELAY_3)
    xb[4].pass_through_delay(3, 5)
    xb[5].enable_alu(_UAluOp.MULTIPLY, _AI.PREV_ALU_OUT, _AI.PREV_DELAY_3)
    xb[5].pass_through_delay(5)
    for k in range(6, 8):
        xb[k].pass_through_alu()
        xb[k].pass_through_delay(5)
    x2.enable_output(_OutSel.DELAY_5, _OutPath.WR0_LO)
    x2.enable_output(_OutSel.ALU_OUT, _OutPath.WR0_HI)
    return [reg], [x2]


def _rny_uops():
    """rn = (alpha_g^2 - q) * alpha_g - 1, 2x with 4 stages per element
    (el0 blk0-3, el1 blk4-7).  Channels: 0=a_lo 1=q_lo 2=a_hi 3=q_hi
    4=ONE 5=park."""
    def base(hi):
        u = _UopConfig()
        u.enable_input(_InpSel.SRC_0, 1)
        u.enable_input(_InpSel.SRC_1, 2)
        if hi:
            u.enable_input(_InpSel.SRC_0_HI, 3)
            u.enable_input(_InpSel.SRC_1_HI, 4)
        u.enable_input(_InpSel.ONE_F32, 5)
        u.require_inp0 = _ENABLE
        u.require_inp1 = _ENABLE
        u.trigger = (_Trigger.SRC_TENSOR_DONE, _Trigger.NONE, _Trigger.NONE)
        u.next_uop = (0, 0, 0)
        return u

    reg = base(False)
    rb = reg.datapath_config
    rb[0].enable_alu(_UAluOp.MULTIPLY, _AI.PREV_DELAY_0, _AI.PREV_DELAY_0)
    rb[0].pass_through_delay(0, 1, 4)
    rb[1].enable_alu(_UAluOp.SUBTRACT, _AI.PREV_ALU_OUT, _AI.PREV_DELAY_1)
    rb[1].pass_through_delay(0, 4)
    rb[2].enable_alu(_UAluOp.MULTIPLY, _AI.PREV_ALU_OUT, _AI.PREV_DELAY_0)
    rb[2].pass_through_delay(4)
    rb[3].enable_alu(_UAluOp.SUBTRACT, _AI.PREV_ALU_OUT, _AI.PREV_DELAY_4)
    for k in range(4, 8):
        rb[k].pass_through_alu()
    reg.enable_output(_OutSel.ALU_OUT, _OutPath.WR0_LO)

    x2 = base(True)
    xb = x2.datapath_config
    xb[0].enable_alu(_UAluOp.MULTIPLY, _AI.PREV_DELAY_0, _AI.PREV_DELAY_0)
    xb[0].pass_through_delay(0, 1, 2, 3, 4)
    xb[1].enable_alu(_UAluOp.SUBTRACT, _AI.PREV_ALU_OUT, _AI.PREV_DELAY_1)
    xb[1].pass_through_delay(0, 2, 3, 4)
    xb[2].enable_alu(_UAluOp.MULTIPLY, _AI.PREV_ALU_OUT, _AI.PREV_DELAY_0)
    xb[2].pass_through_delay(2, 3, 4)
    xb[3].enable_alu(_UAluOp.SUBTRACT, _AI.PREV_ALU_OUT, _AI.PREV_DELAY_4)
    xb[3].pass_through_delay(2, 3, 4)
    # blk4: el1 stage 1 + park rn_lo (blk3's out)
    xb[4].enable_alu(_UAluOp.MULTIPLY, _AI.PREV_DELAY_2, _AI.PREV_DELAY_2)
    xb[4].pass_through_delay(2, 3, 4)
    xb[4].enable_delay_from_src(_DelayInp.PREV_ALU_OUT, 5)
    xb[5].enable_alu(_UAluOp.SUBTRACT, _AI.PREV_ALU_OUT, _AI.PREV_DELAY_3)
    xb[5].pass_through_delay(2, 4, 5)
    xb[6].enable_alu(_UAluOp.MULTIPLY, _AI.PREV_ALU_OUT, _AI.PREV_DELAY_2)
    xb[6].pass_through_delay(4, 5)
    xb[7].enable_alu(_UAluOp.SUBTRACT, _AI.PREV_ALU_OUT, _AI.PREV_DELAY_4)
    xb[7].pass_through_delay(5)
    x2.enable_output(_OutSel.DELAY_5, _OutPath.WR0_LO)
    x2.enable_output(_OutSel.ALU_OUT, _OutPath.WR0_HI)
    return [reg], [x2]


_QSU_REG, _QSU_2X = _qsu_uops()
# q = (alpha_g + 2) * alpha_{g-1}^2     (s0 = 2.0)
OP_QSU = _register_dve_op(
    "TRIDIAG_QSU",
    _Spec(
        body=((_S0 + _One) + _One) * (_S1 * _S1),
        reference=lambda i0, i1, c0, c1, c2: (
            (i0.astype(_np.float32) + 2.0) * i1.astype(_np.float32) ** 2
        ).astype(_np.float32),
    ),
    uops=_QSU_REG,
    uops_2x=_QSU_2X,
)

_RNY_REG, _RNY_2X = _rny_uops()
# rn = (alpha_g^2 - q) * alpha_g - 1   (2-term Neumann for -1/m0)
OP_RNY = _register_dve_op(
    "TRIDIAG_RNY",
    _Spec(
        body=((_S0 * _S0) - _S1) * _S0 - _One,
        reference=lambda i0, i1, c0, c1, c2: (
            (i0.astype(_np.float32) ** 2 - i1.astype(_np.float32))
            * i0.astype(_np.float32) - 1.0
        ).astype(_np.float32),
    ),
    uops=_RNY_REG,
    uops_2x=_RNY_2X,
)


_ASQ_REG, _ASQ_2X = _asq_uops()
# A = alpha_{g-1}^2 * rn
OP_ASQ = _register_dve_op(
    "TRIDIAG_ASQ",
    _Spec(
        body=(_S0 * _S0) * _S1,
        reference=lambda i0, i1, c0, c1, c2: (
            i0.astype(_np.float32) ** 2 * i1
        ).astype(_np.float32),
    ),
    uops=_ASQ_REG,
    uops_2x=_ASQ_2X,
)

_NCP_REG, _NCP_2X = _ncpf_uops()
# ncp = ((alpha_{g+1} + 2) * alpha_{g+1}) * rn    (s0 = 2.0)
OP_NCPF = _register_dve_op(
    "TRIDIAG_NCPF",
    _Spec(
        body=((_S0 + _One) + _One) * _S0 * _S1,
        reference=lambda i0, i1, c0, c1, c2: (
            (i0.astype(_np.float32) + 2.0) * i0 * i1
        ).astype(_np.float32),
    ),
    uops=_NCP_REG,
    uops_2x=_NCP_2X,
)


def _cuts():
    """Chunk cuts in f-col space [0, T2), even, chunk 0 small.  ext[c] is
    the halo-extended range all of chunk c's elementwise tensors and its
    fwd scan cover: warmup HFW below + HBW spill above (so the bwd scan
    of the SAME chunk finds its warmup data locally)."""
    m = ~(S - 1)
    ccut = [0]
    acc = 0.0
    for fr in FRACS:
        acc += fr
        ccut.append(int(T2 * acc) & m)
    ccut[-1] = T2
    ext = []
    for c in range(NH):
        elo = max(ccut[c] - HFW, 0)
        ehi = min(ccut[c + 1] + HBW, T2)
        ext.append((elo, ehi))
    return ccut, ext


def emit_core(tc, alpha_in, f_in, x_out):
    nc = tc.nc
    ccut, ext = _cuts()

    # per-chunk block bases for the halo-extended intermediates
    pb = []
    acc = 0
    for c in range(NH):
        pb.append(acc)
        acc += ext[c][1] - ext[c][0]
    EXT_COLS = acc

    # bwd sub-scans (2 per chunk, hi then lo): (blo, bhi, whi, xbase);
    # scan covers [blo, whi) reversed, body (DMA'd) is [blo, bhi).
    subs = []
    acc = 0
    for c in range(NH):
        clo, chi = ccut[c], ccut[c + 1]
        if c == NH - 1:
            # split the last chunk so the final output DMA piece is small
            mid = ((clo + chi) // 2) & ~(S - 1)
            ranges = ((mid, chi), (clo, mid))
        else:
            ranges = ((clo, chi),)
        pieces = []
        for (blo, bhi) in ranges:
            whi = min(bhi + HBW, T2)
            pieces.append((blo, bhi, whi, acc))
            acc += whi - blo
        subs.append(pieces)
    X_COLS = acc

    with contextlib.ExitStack() as ctx:
        pool = ctx.enter_context(tc.tile_pool(name="w", bufs=1))
        t_alpha = pool.tile([P, TA2], F16, tag="alpha")
        t_fh = pool.tile([P, T2], F16, tag="fh")
        t_q = pool.tile([P, EXT_COLS], F16, tag="q")
        t_rn = pool.tile([P, EXT_COLS], F16, tag="rn")
        t_A = pool.tile([P, EXT_COLS], F16, tag="A")
        t_Bn = pool.tile([P, EXT_COLS], F16, tag="Bn")
        t_ncp = pool.tile([P, EXT_COLS], F16, tag="ncp")
        # 2x mode needs every scan operand 2-byte; dpn fp16 costs ~3e-3 abs
        t_dpn = pool.tile([P, EXT_COLS], F16 if SCAN_2X else F32, tag="dpn")
        t_x = pool.tile([P, X_COLS], F16, tag="x")

        # ---- input DMA: per-queue order [alpha_c, f_c], chunks alternating
        # across the two HW DGE queues ----
        dma_engs = (nc.sync, nc.scalar)
        acut = [0]
        fcut = [0]
        for c in range(NH):
            acut.append(ext[c][1] + 2 * S if c < NH - 1 else TA2)
            fcut.append(ext[c][1] if c < NH - 1 else T2)
        for c in range(NH):
            eng = dma_engs[c % 2]
            eng.dma_start(
                t_alpha[:, acut[c]:acut[c + 1]],
                bass.AP(alpha_in, acut[c], [[TA2, P], [1, acut[c + 1] - acut[c]]]),
            )
            eng.dma_start(
                t_fh[:, fcut[c]:fcut[c + 1]],
                bass.AP(f_in, fcut[c], [[T2, P], [1, fcut[c + 1] - fcut[c]]]),
            )

        # ---- per-chunk pipeline: m0rn -> A,Bn -> fwd -> ncp -> bwd -> DMA ----
        nout = 0
        for c in range(NH):
            elo, ehi = ext[c]
            L = ehi - elo
            b = pb[c]
            q_i = nc.vector._custom_dve(
                OP_QSU,
                out=t_q[:, b:b + L],
                in0=t_alpha[:, elo + S:ehi + S],
                in1=t_alpha[:, elo:ehi],
                s0=2.0,
            )
            q_i.ins.perf_max = 1
            r_i = nc.vector._custom_dve(
                OP_RNY,
                out=t_rn[:, b:b + L],
                in0=t_alpha[:, elo + S:ehi + S],
                in1=t_q[:, b:b + L],
            )
            r_i.ins.perf_max = 1
            a_i = nc.vector._custom_dve(
                OP_ASQ,
                out=t_A[:, b:b + L],
                in0=t_alpha[:, elo:ehi],
                in1=t_rn[:, b:b + L],
            )
            a_i.ins.perf_max = 1
            nc.vector.tensor_tensor(
                t_Bn[:, b:b + L], t_fh[:, elo:ehi], t_rn[:, b:b + L], ALU.mult
            )
            fwd = nc.vector._custom_dve(
                OP_SCANA,
                out=t_dpn[:, b:b + L],
                in0=t_A[:, b:b + L],
                in1=t_Bn[:, b:b + L],
            )
            if SCAN_2X:
                fwd.ins.perf_max = 1
            n_i = nc.vector._custom_dve(
                OP_NCPF,
                out=t_ncp[:, b:b + L],
                in0=t_alpha[:, elo + 2 * S:ehi + 2 * S],
                in1=t_rn[:, b:b + L],
                s0=2.0,
            )
            n_i.ins.perf_max = 1
            for (blo, bhi, whi, xb) in subs[c]:
                Lw = whi - blo
                bwd = nc.vector._custom_dve(
                    OP_SCANS,
                    out=t_x[:, xb:xb + Lw][:, ::-1],
                    in0=t_ncp[:, b + blo - elo:b + whi - elo][:, ::-1],
                    in1=t_dpn[:, b + blo - elo:b + whi - elo][:, ::-1],
                )
                if SCAN_2X:
                    bwd.ins.perf_max = 1
                slo, shi = max(blo, HFW), min(bhi, HFW + S * DH)
                if shi > slo:
                    dma_engs[nout % 2].dma_start(
                        bass.AP(x_out, slo - HFW, [[S * DH, P], [1, shi - slo]]),
                        t_x[:, xb + slo - blo:xb + shi - blo],
                    )
                    nout += 1


def build_nc():
    nc = bacc.Bacc(
        "TRN2", target_bir_lowering=False, debug=False, num_devices=NCORES
    )
    alpha_in = nc.dram_tensor("alpha_in", [P * TA2], F16, kind="ExternalInput")
    f_in = nc.dram_tensor("f_in", [P * T2], F16, kind="ExternalInput")
    x_out = nc.dram_tensor("x_out", [P * S * DH], F16, kind="ExternalOutput")
    with tile.TileContext(nc) as tc:
        emit_core(tc, alpha_in, f_in, x_out)
    nc.compile()
    return nc


def shard_inputs(alpha, f):
    """Window + interleave on the host: per (core, lane, stream) the f
    window covers rows [start-HF, start+DH+HB) and the alpha window
    [start-HF-1, start+DH+HB+1); streams interleave column-wise
    (col = 2*j + s)."""
    alpha_pad = np.zeros(N + WA + DH, dtype=np.float16)
    alpha_pad[HF + 1: HF + 1 + N] = alpha.astype(np.float16)
    f_pad = np.zeros(N + W + DH, dtype=np.float16)
    f_pad[HF: HF + N] = f.astype(np.float16)

    nstreams = N // DH
    aw = np.lib.stride_tricks.sliding_window_view(alpha_pad, WA)[::DH][:nstreams]
    fw = np.lib.stride_tricks.sliding_window_view(f_pad, W)[::DH][:nstreams]
    aw = aw.reshape(NCORES, P, S, WA).transpose(0, 1, 3, 2)
    fw = fw.reshape(NCORES, P, S, W).transpose(0, 1, 3, 2)
    in_maps = []
    for c in range(NCORES):
        in_maps.append(
            {
                "alpha_in": np.ascontiguousarray(aw[c]).reshape(-1),
                "f_in": np.ascontiguousarray(fw[c]).reshape(-1),
            }
        )
    return in_maps


def unshard_output(results):
    out = np.empty((NCORES, P, DH, S), dtype=np.float16)
    for c in range(NCORES):
        out[c] = results[c]["x_out"].reshape(P, DH, S)
    return out.transpose(0, 1, 3, 2).reshape(-1).astype(np.float32)


_NC_CACHE = {}


def kernel(alpha: np.ndarray, f: np.ndarray, trace: bool = False, **run_kwargs):
    from concourse import bass_utils

    alpha = np.asarray(alpha, dtype=np.float32)
    f = np.asarray(f, dtype=np.float32)
    assert alpha.shape == (N,) and f.shape == (N,)
    key = (tuple(FRACS), HF, HB, S, POOL_PRODUCTS)
    if key not in _NC_CACHE:
        _NC_CACHE[key] = build_nc()
    nc = _NC_CACHE[key]
    in_maps = shard_inputs(alpha, f)
    res = bass_utils.run_bass_kernel_spmd(
        nc, in_maps, core_ids=list(range(NCORES)), trace=trace, **run_kwargs
    )
    out = unshard_output(res.results)
    if trace:
        kernel.last_results = res
    return out
